# revision 22
# baseline (speedup 1.0000x reference)
"""Trainium2 Bass kernel for nn_BasicS2ConvV2.

out[b,d,p,r] = sum_{c,k,a} Wfull[d,c,r,k,a] * x[b,c,k,p,a]
with Wfull gathered on host from the 36 free params (tiny), and the
31.4 GFLOP contraction run on 8 NeuronCores, data-parallel over b.

Per-core device problem: o[p=4096, dr=192] = xs[cka, p]^T @ WT[cka, dr]
with cka = 16*13*12 = 2496 padded to 2560 = 20 k-tiles of 128.

Transposed mapping (vs W-stationary): the x k-tile [128k x 128p] is the
stationary operand and the W k-tile [128k x 192dr] is the moving one, so
every matmul streams 192 columns with the full 128x128 array utilized:
32 p-tiles x 20 k-tiles x 192 = 122,880 PE cycles/core vs 163,840 for
the W-stationary schedule (dr=192 forces a half-empty 64-wide pass).

x ships as fp8 e3m4 (4 mantissa bits) and W as bf16 (mixed-dtype matmul
verified exact on HW); PSUM accumulates fp32. NBF k-tiles of x can be
kept in bf16 for extra accuracy margin if needed (NBF=0: measured rel
err ~1.3e-2 vs the 2e-2 gate). fp8 keeps the kernel PE-bound rather
than DMA-bound.

DMAs are batched G=4 p-tiles per transfer: descriptor generation
(HWDGE) costs ~630ns per DMA instruction regardless of size, so fewer,
larger transfers keep the DGE off the critical path. The first group is
pt-granular and interleaved with the four wt chunks so the matmul
pipeline starts sooner; dummy warmup matmuls on scratch data burn the
PE p-state ramp window during the initial fill.

Host pre-layout makes every device DMA a fully sequential HBM stream:
  xs8: [NG, 128, G, T8, 128] fp8   (contiguous per group)
  xsb: [NG, 128, G, NBF, 128] bf16 (only if NBF > 0)
  wt:  [128, KT, DR] bf16          (4 chunked DMAs, loaded once)
  o:   [NG, 128, G, DR] f16        (one contiguous block per group)
"""

import numpy as np
import ml_dtypes

B, C, KS, P, A = 8, 16, 13, 4096, 12
D, R = 16, 12
CKA = C * KS * A          # 2496
KT = 20                   # contraction tiles of 128 (2560 padded)
CKA_PAD = KT * 128
DR = D * R                # 192
NPT = P // 128            # 32 p-tiles
NBF = 0                   # k-tiles kept in bf16 (accuracy headroom knob)
T8 = KT - NBF             # k-tiles shipped as fp8 e3m4
G = 4                     # p-tiles per DMA group
NG = NPT // G             # 8 groups
NWARM = 14                # PE warmup dummy matmuls

MMDT = "mixed"            # kept for test.py compat

_cache = {}


def _emit_body(nc, xs8, xsb, wtile, wt, o, o_dt, xpool, opool, pspool, reps,
               do_dma=True, do_mm=True, do_out=True, xt_static=None,
               wt_whole=False):
    import concourse.mybir as mybir

    seq = [(r, g) for r in range(reps) for g in range(NG)]
    for r, g in seq:
        first = (r, g) == (0, 0)
        last = (r, g) == seq[-1]
        if do_dma:
            xt8 = xpool.tile([128, G, T8, 128], mybir.dt.float8e3, tag="xt8")
            xtb = (xpool.tile([128, G, NBF, 128], mybir.dt.bfloat16, tag="xtb")
                   if NBF else None)
            if first and not wt_whole:
                # pt-granular first fills interleaved with the wt chunks so
                # the matmul pipeline ramps ~4us sooner
                for j in range(G):
                    nc.sync.dma_start(wtile[:, j * 5:(j + 1) * 5, :],
                                      wt[:, j * 5:(j + 1) * 5, :])
                    nc.scalar.dma_start(xt8[:, j], xs8[g, :, j])
                    if NBF:
                        nc.scalar.dma_start(xtb[:, j], xsb[g, :, j])
            else:
                nc.scalar.dma_start(xt8[:], xs8[g])
                if NBF:
                    nc.scalar.dma_start(xtb[:], xsb[g])
        else:
            xt8, xtb = xt_static
        if not do_mm:
            continue
        ot = opool.tile([128, G, DR], o_dt, tag="ot")
        for j in range(G):
            ps = pspool.tile([128, DR], mybir.dt.float32, tag="ps")
            for t in range(T8):
                nc.tensor.matmul(ps[:], xt8[:, j, t, :], wtile[:, t, :],
                                 start=(t == 0), stop=(t == KT - 1))
            for i in range(NBF):
                t = T8 + i
                nc.tensor.matmul(ps[:], xtb[:, j, i, :], wtile[:, t, :],
                                 start=(t == 0), stop=(t == KT - 1))
            if do_out:
                nc.vector.tensor_copy(ot[:, j, :], ps[:])
                if last and j == G - 2:
                    # drain the first 3 p-tiles early to shorten the tail
                    nc.sync.dma_start(o[g, :, 0:G - 1], ot[:, 0:G - 1])
        if do_out:
            if last:
                nc.sync.dma_start(o[g, :, G - 1:G], ot[:, G - 1:G])
            else:
                nc.sync.dma_start(o[g], ot[:])


def _build_program(mmdt=None, reps=1, loop_n=0, do_dma=True, do_mm=True,
                   do_out=True, internal_io=False):
    import concourse.bacc as bacc
    import concourse.mybir as mybir
    from concourse.tile import TileContext
    from contextlib import nullcontext

    f8 = mybir.dt.float8e3
    bf = mybir.dt.bfloat16
    o_dt = mybir.dt.float16
    nbf = max(NBF, 1)  # dram decl needs a nonzero dim; unused when NBF=0

    nc = bacc.Bacc("TRN2", target_bir_lowering=False, debug=False)
    if internal_io:
        # Timing-probe build: no host I/O traffic; data is device garbage.
        xs8 = nc.dram_tensor("xs8", [NG, 128, G, T8, 128], f8).ap()
        xsb = nc.dram_tensor("xsb", [NG, 128, G, nbf, 128], bf).ap()
        wt = nc.dram_tensor("wt", [128, KT, DR], bf).ap()
        o = nc.dram_tensor("o", [NG, 128, G, DR], o_dt).ap()
        dume = nc.declare_dram_parameter(
            "dume", [1, 8], mybir.dt.float32, isOutput=True)
    else:
        xs8 = nc.declare_dram_parameter(
            "xs8", [NG, 128, G, T8, 128], f8, isOutput=False)
        xsb = (nc.declare_dram_parameter(
            "xsb", [NG, 128, G, nbf, 128], bf, isOutput=False)
            if NBF else None)
        wt = nc.declare_dram_parameter("wt", [128, KT, DR], bf, isOutput=False)
        o = nc.declare_dram_parameter(
            "o", [NG, 128, G, DR], o_dt, isOutput=True)

    with TileContext(nc) as tc:
        with (
            tc.tile_pool(name="wpool", bufs=1) as wpool,
            tc.tile_pool(name="xpool", bufs=3) as xpool,
            tc.tile_pool(name="opool", bufs=3) as opool,
            tc.tile_pool(name="pspool", bufs=6, space="PSUM") as pspool,
            tc.tile_pool(name="wmpool", bufs=1, space="PSUM") as wmpool,
        ):
            wtile = wpool.tile([128, KT, DR], bf)

            # Warmup: dummy matmuls on scratch data occupy the PE during the
            # initial DMA fill so the p-state ramp window burns on
            # otherwise-idle time. Results land in a scratch PSUM bank and
            # are never read.
            if NWARM and do_mm:
                scr = wpool.tile([128, 256], bf, tag="scr")
                nc.vector.memset(scr[:], 0.25)
                psw = wmpool.tile([128, 256], mybir.dt.float32, tag="wm")
                for _ in range(NWARM):
                    nc.tensor.matmul(psw[:], scr[:, 0:128], scr[:],
                                     start=True, stop=True)

            xt_static = None
            wt_whole = bool(loop_n) or not do_dma
            if wt_whole:
                nc.sync.dma_start(wtile[:], wt[:])
            if not do_dma:
                x8s = wpool.tile([128, G, T8, 128], f8, tag="x8s")
                xbs = wpool.tile([128, G, nbf, 128], bf, tag="xbs")
                nc.any.memset(x8s[:], 0.25)
                nc.any.memset(xbs[:], 0.25)
                xt_static = (x8s, xbs)

            loop_cm = tc.For_i(0, loop_n, 1) if loop_n else nullcontext()
            with loop_cm:
                _emit_body(nc, xs8, xsb, wtile, wt, o, o_dt,
                           xpool, opool, pspool, reps,
                           do_dma=do_dma, do_mm=do_mm, do_out=do_out,
                           xt_static=xt_static, wt_whole=wt_whole)

            if internal_io:
                dtile = opool.tile([1, 8], mybir.dt.float32, tag="dume")
                nc.any.memset(dtile[:], 1.0)
                nc.sync.dma_start(dume[:], dtile[:])

    nc.compile()
    return nc


def _get_program():
    if "main" not in _cache:
        _cache["main"] = _build_program()
    return _cache["main"]


def _prep_inputs(x, W, idx_map, tivr, tir):
    """Host prep: weight gather + relayout to sequential-DMA order."""
    Wm = W[:, :, idx_map].reshape(D, C, KS, A)
    Wfull = Wm[:, :, tivr[:, :, None], tir[:, None, :]]       # [d,c,r,k,a]
    WT = Wfull.transpose(1, 3, 4, 0, 2).reshape(CKA, DR)      # [(c,k,a),(d,r)]
    WT_pad = np.zeros((CKA_PAD, DR), dtype=np.float32)
    WT_pad[:CKA] = WT
    # [2560, DR] -> [128(q), KT(t), DR]
    wt_q = np.ascontiguousarray(
        WT_pad.reshape(KT, 128, DR).transpose(1, 0, 2)).astype(ml_dtypes.bfloat16)

    # x[b,c,k,p,a] -> [b, (c,k,a), p] -> [b, NG, 128(q), G, KT(t), 128(p)]
    xt = np.ascontiguousarray(x.transpose(0, 1, 2, 4, 3)).reshape(B, CKA, P)
    xs_pad = np.zeros((B, CKA_PAD, P), dtype=np.float32)
    xs_pad[:, :CKA] = xt
    xs_q = np.ascontiguousarray(
        xs_pad.reshape(B, KT, 128, NG, G, 128).transpose(0, 3, 2, 4, 1, 5))
    xs8 = np.ascontiguousarray(
        xs_q[:, :, :, :, :T8]).astype(ml_dtypes.float8_e3m4)
    xsb = (np.ascontiguousarray(
        xs_q[:, :, :, :, T8:]).astype(ml_dtypes.bfloat16) if NBF else None)
    return xs8, xsb, wt_q


def kernel(x, W, idx_map, trace_idxv_rot, trace_idx_rot):
    from concourse.bass_utils import run_bass_kernel_spmd

    x = np.asarray(x)
    W = np.asarray(W, dtype=np.float32)
    idx_map = np.asarray(idx_map)
    tivr = np.asarray(trace_idxv_rot)
    tir = np.asarray(trace_idx_rot)

    xs8, xsb, wt_q = _prep_inputs(x, W, idx_map, tivr, tir)

    nc = _get_program()
    in_maps = [{"xs8": xs8[b], "wt": wt_q} for b in range(B)]
    if NBF:
        for b in range(B):
            in_maps[b]["xsb"] = xsb[b]
    res = run_bass_kernel_spmd(nc, in_maps, list(range(B)))

    out = np.empty((B, D, P, R), dtype=np.float32)
    for b in range(B):
        oraw = res.results[b]["o"].astype(np.float32)  # [NG, 128, G, DR]
        ob = oraw.transpose(0, 2, 1, 3).reshape(P, D, R)
        out[b] = ob.transpose(1, 0, 2)
    return out
